# revision 42
# baseline (speedup 1.0000x reference)
"""Trainium2 Bass kernel for the dual cross-attention similarity module.

Math (per query q of 300, way w of 5):
  qkv from shared W; B->A attention (25 b-tokens over 125 a-tokens/way) and
  A->B attention (125 a-tokens/way over 25 b-tokens); outputs are negated
  squared Frobenius distances between v and softmax-reconstructions.

Sharding: queries split 40/core across 8 cores (300 padded to 320);
features_a / W / consts replicated; no collectives.

Per-core design (v3):
  * Inputs quantized host-side to fp8e4m3 (W pre-scaled x2; outputs
    unscaled /WS^2 in the final device ops) -> 1.3MB load.
  * qkv runs as fp8 DoubleRow matmuls over a 768-padded contraction.
  * q/k/v are retiled to [64, 2, T] fp8 via sbuf->sbuf DMAs so the score
    and cross-gram matmuls also run DoubleRow at 0.5 cyc/row (stationaries
    padded to even widths: a-ways at 128, b-groups via [64,2,8,128]).
  * n_a / n_b fold into Z-independent terms: n_b rides as row 96 of the
    B accumulator bank (through the same evac + transpose path as Z/C1/C2);
    n_a rides as row 96 of phase C's V bank.
  * B->A per 500-col chunk: 5 ways of DR scores in two psum tiles, fused
    exps; R = V_a E; Z/c1/c2 thin-reduced into one psum bank (rows
    0:5/32:37/64:69). Tail: rows transposed to [128 l, 8, 97] slots,
    rational math on 40-col tiles, indicator matmuls -> [40, 6].
  * A->B: way-aligned a-chunks (384, 256); per group: DR scores + DR
    cross-gram, aeb = blockdiag-gram @ E, Z/U/V accumulated via bo40;
    tail reads psum directly with way-aligned 125-slices.  The B tail is
    emitted between the two C chunks to overlap its serial latency.
"""

import numpy as np
import ml_dtypes

import concourse.bass as bass
import concourse.bacc as bacc
import concourse.tile as tile
from concourse import mybir
from concourse.bass_utils import run_bass_kernel_spmd

F32 = mybir.dt.float32
BF16 = mybir.dt.bfloat16
FP8 = mybir.dt.float8e4
AL = mybir.AluOpType
AF = mybir.ActivationFunctionType
AX = mybir.AxisListType
DR = mybir.MatmulPerfMode.DoubleRow

WS = 2.0                      # host-side W scale (outputs carry WS^2)
SCALE = 0.08838834764831845 / (WS * WS)   # 1/sqrt(128) / WS^2
N_CORES = 8
NQ = 40          # queries per core
LB = 25          # b tokens per query
LA = 125         # a tokens per way
NW = 5
TB = NQ * LB     # 1000
TAP = NW * 128   # 640 (a tokens padded to 128/way)

# consts blob (bf16) column offsets
C_ID = 0          # ident [128,128]
C_OW5 = 128       # [125,5,5]
C_OW128 = 153     # [128,5,5]
C_BO40 = 178      # [125,8,40]
C_BDM = 498       # [125,125]
C_ONES = 623      # [128,1]
C_ONES40 = 624    # [1,40]
NCB = 664
# f32 blob offsets
F_IND = 0         # [128,8,40]
F_ID = 320        # [128,128]
NCF = 448


def build_nc():
    nc = bacc.Bacc("TRN2", target_bir_lowering=False, debug=False)

    fa_d = nc.dram_tensor("fa", [640, TAP], FP8, kind="ExternalInput")
    fb_d = nc.dram_tensor("fb", [640, TB], FP8, kind="ExternalInput")
    wt_d = nc.dram_tensor("wt", [640, 384], FP8, kind="ExternalInput")
    cb_d = nc.dram_tensor("cb", [128, NCB], BF16, kind="ExternalInput")
    cf_d = nc.dram_tensor("cf", [128, NCF], F32, kind="ExternalInput")
    sq_d = nc.dram_tensor("sq", [NQ, NW], F32, kind="ExternalOutput")
    qs_d = nc.dram_tensor("qs", [NQ, NW], F32, kind="ExternalOutput")

    with tile.TileContext(nc) as tc:
        with (
            tc.tile_pool(name="const", bufs=1) as const,
            tc.tile_pool(name="feat", bufs=1) as feat,
            tc.tile_pool(name="persist", bufs=1) as persist,
            tc.tile_pool(name="ew", bufs=1) as ew,
            tc.tile_pool(name="work", bufs=2) as work,
        ):
            # ---------------- loads ----------------
            wt = feat.tile([128, 6, 384], FP8)
            nc.gpsimd.memset(wt[:, 5, :], 0.0)
            nc.sync.dma_start(out=wt[:, 0:5, :],
                              in_=wt_d.rearrange("(cb c) e -> c cb e", c=128))
            fa = feat.tile([128, 6, TAP], FP8)
            nc.gpsimd.memset(fa[:, 5, :], 0.0)
            nc.sync.dma_start(out=fa[:, 0:5, :],
                              in_=fa_d.rearrange("(cb c) t -> c cb t", c=128))
            fb = feat.tile([128, 6, TB], FP8)
            nc.gpsimd.memset(fb[:, 5, :], 0.0)
            nc.sync.dma_start(out=fb[:, 0:5, :],
                              in_=fb_d.rearrange("(cb c) t -> c cb t", c=128))
            cb = const.tile([128, NCB], BF16)
            nc.sync.dma_start(out=cb, in_=cb_d[:])
            cf = const.tile([128, NCF], F32)
            nc.sync.dma_start(out=cf, in_=cf_d[:])
            ident = cb[:, C_ID:C_ID + 128]
            ow5 = cb[0:125, C_OW5:C_OW5 + 25].rearrange("p (w c) -> p w c", w=5)
            ow128 = cb[:, C_OW128:C_OW128 + 25].rearrange("p (w c) -> p w c", w=5)
            bo40 = cb[0:125, C_BO40:C_BO40 + 320].rearrange("p (g c) -> p g c", g=8)
            bdm = cb[0:125, C_BDM:C_BDM + 125]
            ones128 = cb[:, C_ONES:C_ONES + 1]
            ones40 = cb[0:1, C_ONES40:C_ONES40 + 40]
            indB = cf[:, F_IND:F_IND + 320].rearrange("p (g c) -> p g c", g=8)
            identf = cf[:, F_ID:F_ID + 128]

            # warm ACT tables (exp/copy/square live in one set)
            warm = work.tile([1, 1], F32, tag="warm")
            nc.scalar.activation(out=warm, in_=cb[0:1, 0:1], func=AF.Exp)

            # ---------------- phase A: qkv ----------------
            qa8 = persist.tile([128, TAP], FP8)
            ka8 = persist.tile([128, TAP], FP8)
            vaB = persist.tile([128, TAP], BF16)
            va8 = persist.tile([128, TAP], FP8)
            qb8 = persist.tile([128, TB], FP8)
            kb8 = persist.tile([128, TB], FP8)
            vbB = persist.tile([128, TB], BF16)
            vb8 = persist.tile([128, TB], FP8)

            psB1_cm = tc.tile_pool(name="psB1", bufs=1, space="PSUM")
            psB1 = psB1_cm.__enter__()
            psA_cm = tc.tile_pool(name="psA", bufs=1, space="PSUM")
            psA = psA_cm.__enter__()

            def qkv_emit(src, chunks, dests):
                for e in range(3):
                    done = 0
                    for grp in chunks:
                        gw = sum(grp)
                        pq = psA.tile([128, 512], F32, tag="pqkv", bufs=2)
                        off = 0
                        for cw in grp:
                            c0 = done + off
                            for j in range(3):
                                nc.tensor.matmul(
                                    pq[:, off:off + cw],
                                    wt[:, 2 * j:2 * j + 2, e * 128:(e + 1) * 128],
                                    src[:, 2 * j:2 * j + 2, c0:c0 + cw],
                                    start=(j == 0), stop=(j == 2),
                                    perf_mode=DR,
                                )
                            off += cw
                        for k, dst in enumerate(dests[e]):
                            if e == 0 or (e == 2 and k == 0):
                                nc.scalar.copy(out=dst[:, done:done + gw],
                                               in_=pq[:, 0:gw])
                            else:
                                nc.vector.tensor_copy(out=dst[:, done:done + gw],
                                                      in_=pq[:, 0:gw])
                        done += gw

            qkv_emit(fa, [[256, 256], [128]], [[qa8], [ka8], [vaB, va8]])
            # B-critical retile first
            kaD = persist.tile([64, 2, TAP], FP8)
            nc.sync.dma_start(out=kaD[:, 0, :], in_=ka8[0:64, :])
            nc.sync.dma_start(out=kaD[:, 1, :], in_=ka8[64:128, :])
            # vA in token-major via transposes (rp stationary)
            vA_tok = persist.tile([125, 5, 128], BF16)
            ptp = psA.tile([125, 5, 128], BF16, tag="ptp", bufs=1)
            for w in range(5):
                nc.tensor.transpose(ptp[:, w, :],
                                    vaB[:, w * 128:w * 128 + 125], ident)
            nc.vector.tensor_copy(out=vA_tok, in_=ptp)
            sqa = work.tile([128, TAP], BF16, tag="sqa")
            nc.gpsimd.tensor_mul(sqa, vaB, vaB)

            qkv_emit(fb, [[250, 250], [250, 250]], [[qb8], [kb8], [vbB, vb8]])
            qbD = persist.tile([64, 2, TB], FP8)
            nc.sync.dma_start(out=qbD[:, 0, :], in_=qb8[0:64, :])
            nc.sync.dma_start(out=qbD[:, 1, :], in_=qb8[64:128, :])
            qaD = persist.tile([64, 2, TAP], FP8)
            vaD = persist.tile([64, 2, TAP], FP8)
            for s8, dst in ((qa8, qaD), (va8, vaD)):
                nc.sync.dma_start(out=dst[:, 0, :], in_=s8[0:64, :])
                nc.sync.dma_start(out=dst[:, 1, :], in_=s8[64:128, :])
            kbD = persist.tile([64, 2, 8, 128], FP8)
            vbD = persist.tile([64, 2, 8, 128], FP8)
            for s8, dst in ((kb8, kbD), (vb8, vbD)):
                nc.gpsimd.memset(dst[:, :, :, 125:128], 0.0)
                nc.sync.dma_start(
                    out=dst[:, 0, :, 0:125],
                    in_=s8[0:64, :].rearrange("p (g t) -> p g t", g=8))
                nc.sync.dma_start(
                    out=dst[:, 1, :, 0:125],
                    in_=s8[64:128, :].rearrange("p (g t) -> p g t", g=8))
            sqb = work.tile([128, TB], BF16, tag="sqb")
            nc.vector.tensor_mul(sqb, vbB, vbB)
            nbS = persist.tile([1, 1024], F32)
            nc.gpsimd.memset(nbS[:, TB:1024], 0.0)

            # ---------------- phase B: B attends A ----------------
            # scores pool (psB1) is already open; emit scores for chunk 0
            # before phase A's psum pool closes.
            zs = persist.tile([69, 1024], F32)
            nc.gpsimd.memset(zs[:, TB:1024], 1.0)
            sbaA = psB1.tile([128, 2, 512], F32, tag="sbaA", bufs=1)
            sbaB = psB1.tile([128, 3, 512], F32, tag="sbaB", bufs=1)

            def emit_scores(c0):
                for w in range(5):
                    dst = sbaA[:, w, :] if w < 2 else sbaB[:, w - 2, :]
                    for h in range(2):
                        nc.tensor.matmul(
                            dst[:, h * 250:(h + 1) * 250],
                            kaD[:, :, w * 128:(w + 1) * 128],
                            qbD[:, :, c0 + h * 250:c0 + (h + 1) * 250],
                            start=True, stop=True, perf_mode=DR)

            emit_scores(0)
            psA_cm.__exit__(None, None, None)
            psB_cm = tc.tile_pool(name="psB", bufs=1, space="PSUM")
            psB = psB_cm.__enter__()
            zcc = psB.tile([128, 512], F32, tag="zcc", bufs=1)
            for ci in range(2):
                c0 = ci * 500
                if ci == 1:
                    emit_scores(c0)
                eA = ew.tile([128, 2, 500], BF16, tag="eA", bufs=1)
                nc.scalar.activation(out=eA, in_=sbaA[:, :, 0:500],
                                     func=AF.Exp, scale=SCALE)
                eB = ew.tile([128, 3, 500], BF16, tag="eB", bufs=1)
                nc.scalar.activation(out=eB, in_=sbaB[:, :, 0:500],
                                     func=AF.Exp, scale=SCALE)
                for w in range(5):
                    e_w = (eA[:, w, :] if w < 2 else eB[:, w - 2, :])[0:125, :]
                    rp = psB.tile([128, 512], F32, tag="rp", bufs=1)
                    nc.tensor.matmul(rp[:, 0:500], vA_tok[:, w, :], e_w,
                                     start=True, stop=True)
                    c1sb = work.tile([128, 500], BF16, tag="c1sb", bufs=2)
                    nc.vector.tensor_mul(c1sb, rp[:, 0:500],
                                         vbB[:, c0:c0 + 500])
                    c2sb = work.tile([128, 500], BF16, tag="c2sb", bufs=2)
                    nc.scalar.activation(out=c2sb, in_=rp[:, 0:500],
                                         func=AF.Square)
                    nc.tensor.matmul(zcc[0:5, 0:500], ow5[:, w, :], e_w,
                                     start=(w == 0), stop=(w == 4))
                    nc.tensor.matmul(zcc[32:37, 0:500], ow128[:, w, :], c1sb,
                                     start=(w == 0), stop=(w == 4))
                    nc.tensor.matmul(zcc[64:69, 0:500], ow128[:, w, :], c2sb,
                                     start=(w == 0), stop=(w == 4))
                nc.scalar.copy(out=zs[0:69, c0:c0 + 500], in_=zcc[0:69, 0:500])
                pnb = psB.tile([1, 512], F32, tag="pnb", bufs=1,
                               padded_shape=[128, 512])
                nc.tensor.matmul(pnb[:, 0:500], ones128,
                                 sqb[:, c0:c0 + 500], start=True, stop=True)
                nc.vector.tensor_copy(out=nbS[:, c0:c0 + 500],
                                      in_=pnb[:, 0:500])
            psB_cm.__exit__(None, None, None)
            psB1_cm.__exit__(None, None, None)

            # ---------------- phase C (+ B tail interleaved) ----------------
            psC_cm = tc.tile_pool(name="psC", bufs=1, space="PSUM")
            psC = psC_cm.__enter__()
            psT_cm = tc.tile_pool(name="psT", bufs=1, space="PSUM")
            psT = psT_cm.__enter__()

            misc = psT.tile([128, 512], F32, tag="misc", bufs=1)

            def emit_b_tail():
                # zs -> [128 l, 8 slice, 97] transposed slots
                TtS = persist.tile([128, 8, 70], F32)
                for half in range(2):
                    Tt = psT.tile([128, 4, 128], F32, tag="Tt", bufs=1)
                    for sl in range(4):
                        s = half * 4 + sl
                        nc.tensor.transpose(Tt[:, sl, 0:69],
                                            zs[:, s * 128:(s + 1) * 128],
                                            identf[0:69, 0:69])
                        nc.tensor.transpose(Tt[:, sl, 69:70],
                                            nbS[:, s * 128:(s + 1) * 128],
                                            identf[0:1, 0:1])
                    nc.scalar.copy(out=TtS[:, half * 4:half * 4 + 4, 0:70],
                                   in_=Tt[:, :, 0:70])
                rT = work.tile([128, 8, 5], F32, tag="rT")
                nc.vector.reciprocal(out=rT, in_=TtS[:, :, 0:5])
                u1T = work.tile([128, 8, 5], F32, tag="u1T")
                nc.gpsimd.tensor_mul(u1T, TtS[:, :, 64:69], rT)
                t3 = work.tile([128, 8, 5], F32, tag="t3")
                nc.vector.scalar_tensor_tensor(out=t3, in0=TtS[:, :, 32:37],
                                               scalar=-2.0, in1=u1T,
                                               op0=AL.mult, op1=AL.add)
                nc.gpsimd.tensor_mul(TtS[:, :, 64:69], t3, rT)
                qsp = misc[0:40, 0:6]
                for s in range(8):
                    nc.tensor.matmul(qsp, indB[:, s, :], TtS[:, s, 64:70],
                                     start=(s == 0), stop=(s == 7))
                qsb = work.tile([40, 6], F32, tag="qsb")
                nc.vector.tensor_copy(out=qsb, in_=qsp)
                qs_sb = work.tile([40, 5], F32, tag="qs_sb")
                nc.vector.tensor_scalar(
                    out=qs_sb, in0=qsb[:, 0:5], scalar1=qsb[:, 5:6],
                    scalar2=-1.0 / (WS * WS), op0=AL.add, op1=AL.mult)
                nc.sync.dma_start(out=qs_d[:], in_=qs_sb)

            # blockdiag gram of v_b per group, using the gab-tagged banks
            bds = persist.tile([125, 8, 125], BF16)
            for g in range(8):
                gsl = slice(g * 125, (g + 1) * 125)
                pgram = psC.tile([128, 512], F32, tag="gab", bufs=2)
                nc.tensor.matmul(pgram[0:125, 0:125], vb8[:, gsl], vb8[:, gsl],
                                 start=True, stop=True)
                nc.vector.tensor_mul(bds[:, g, :], pgram[0:125, 0:125], bdm)

            sq_parts = work.tile([40, 5], F32, tag="sq_parts")
            naB = persist.tile([1, 5], BF16)
            napb_ap = None
            mul_ctr = [0]
            for ci, (c0, cn) in enumerate(((0, 384), (384, 256))):
                nwc = cn // 128
                zuv = psC.tile([128, 512], F32, tag="zuv", bufs=1)
                vp = psC.tile([128, 512], F32, tag="vp", bufs=1)
                # zuv rows: 0:40 Z, 64:104 U ; vp rows: 0:40 V, 64:104 na-b,
                # 96 (via tile_position) n_a row
                for gp in range(4):
                    sab = psC.tile([128, 2, 512], F32, tag="sab", bufs=1)
                    for gg in range(2):
                        g = gp * 2 + gg
                        for h in range(2):
                            h0, hn = h * (cn // 2), cn // 2
                            nc.tensor.matmul(
                                sab[:, gg, h0:h0 + hn],
                                kbD[:, :, g, :],
                                qaD[:, :, c0 + h0:c0 + h0 + hn],
                                start=True, stop=True, perf_mode=DR)
                    eg2 = ew.tile([128, 2, 384], BF16, tag="eg2", bufs=2)
                    nc.scalar.activation(out=eg2[:, :, 0:cn],
                                         in_=sab[:, :, 0:cn],
                                         func=AF.Exp, scale=SCALE)
                    for gg in range(2):
                        g = gp * 2 + gg
                        e_g = eg2[0:125, gg, 0:cn]
                        gab = psC.tile([128, 512], F32, tag="gab", bufs=2)
                        for h in range(2):
                            h0, hn = h * (cn // 2), cn // 2
                            nc.tensor.matmul(
                                gab[:, h0:h0 + hn], vbD[:, :, g, :],
                                vaD[:, :, c0 + h0:c0 + h0 + hn],
                                start=True, stop=True, perf_mode=DR)
                        egp = work.tile([125, 384], BF16, tag="egp", bufs=2)
                        mul_ctr[0] += 1
                        if mul_ctr[0] % 3 == 0:
                            # shed DVE: evac via Act, multiply on Pool
                            gabS = work.tile([125, 384], BF16, tag="gabS",
                                             bufs=2)
                            nc.scalar.copy(out=gabS[:, 0:cn],
                                           in_=gab[0:125, 0:cn])
                            nc.gpsimd.tensor_mul(egp[:, 0:cn], e_g,
                                                 gabS[:, 0:cn])
                        else:
                            nc.vector.tensor_mul(egp[:, 0:cn], e_g,
                                                 gab[0:125, 0:cn])
                        aeb = psC.tile([128, 512], F32, tag="gab", bufs=2)
                        nc.tensor.matmul(aeb[0:125, 0:cn], bds[:, g, :], e_g,
                                         start=True, stop=True)
                        eab = work.tile([125, 384], BF16, tag="eab", bufs=2)
                        nc.vector.tensor_mul(eab[:, 0:cn], e_g,
                                             aeb[0:125, 0:cn])
                        nc.tensor.matmul(zuv[0:40, 0:cn], bo40[:, g, :], e_g,
                                         start=(g == 0), stop=(g == 7))
                        nc.tensor.matmul(zuv[64:104, 0:cn], bo40[:, g, :],
                                         egp[:, 0:cn],
                                         start=(g == 0), stop=(g == 7))
                        nc.tensor.matmul(vp[0:40, 0:cn], bo40[:, g, :],
                                         eab[:, 0:cn],
                                         start=(g == 0), stop=(g == 7))
                # n_a row for this chunk + per-way reduce
                na_row = misc[64:65, 0:cn]
                nc.tensor.matmul(na_row, ones128,
                                 sqa[:, c0:c0 + cn], start=True, stop=True)
                naF = work.tile([1, 5], F32, tag="naF", bufs=2)
                nc.vector.tensor_reduce(
                    out=naF[:, 0:nwc],
                    in_=na_row.rearrange("p (w l) -> p w l", w=nwc),
                    op=AL.add, axis=AX.X)
                nc.scalar.copy(out=naB[:, ci * 3:ci * 3 + nwc],
                               in_=naF[:, 0:nwc])
                if ci == 1:
                    napb_ap = misc[0:40, 8:13]
                    nc.tensor.matmul(napb_ap, ones40, naB,
                                     start=True, stop=True)
                # tail: f = (2U - V r) r summed over each way's 125 cols
                rab = work.tile([40, 384], F32, tag="rab")
                nc.vector.reciprocal(out=rab[:, 0:cn], in_=zuv[0:40, 0:cn])
                u1 = work.tile([40, 384], F32, tag="u1")
                nc.vector.tensor_mul(u1[:, 0:cn], vp[0:40, 0:cn],
                                     rab[:, 0:cn])
                u2 = work.tile([40, 384], F32, tag="u2")
                nc.vector.scalar_tensor_tensor(
                    out=u2[:, 0:cn], in0=zuv[64:104, 0:cn], scalar=2.0,
                    in1=u1[:, 0:cn], op0=AL.mult, op1=AL.subtract)
                f2 = work.tile([40, 384], F32, tag="f2")
                nc.gpsimd.tensor_mul(f2[:, 0:cn], u2[:, 0:cn], rab[:, 0:cn])
                nc.vector.tensor_reduce(
                    out=sq_parts[:, ci * 3:ci * 3 + nwc],
                    in_=f2[:, 0:cn].rearrange(
                        "p (w l) -> p w l", w=nwc)[:, :, 0:125],
                    op=AL.add, axis=AX.X)
                if ci == 0:
                    emit_b_tail()
            sqt = work.tile([40, 5], F32, tag="sqt")
            nc.vector.tensor_sub(sqt, sq_parts, napb_ap)
            sq_sb = work.tile([40, 5], F32, tag="sq_sb")
            nc.scalar.activation(out=sq_sb, in_=sqt, func=AF.Copy,
                                 scale=1.0 / (WS * WS))
            nc.sync.dma_start(out=sq_d[:], in_=sq_sb)
            psT_cm.__exit__(None, None, None)
            psC_cm.__exit__(None, None, None)

    nc.compile()
    return nc


_CACHE = {}


def _get_nc():
    if "nc" not in _CACHE:
        _CACHE["nc"] = build_nc()
    return _CACHE["nc"]


def _consts():
    cb = np.zeros((128, NCB), np.float32)
    cb[:, C_ID:C_ID + 128] = np.eye(128)
    ow5 = np.zeros((125, 5, 5), np.float32)
    ow128 = np.zeros((128, 5, 5), np.float32)
    for w in range(5):
        ow5[:, w, w] = 1.0
        ow128[:, w, w] = 1.0
    cb[0:125, C_OW5:C_OW5 + 25] = ow5.reshape(125, 25)
    cb[:, C_OW128:C_OW128 + 25] = ow128.reshape(128, 25)
    bo125 = np.kron(np.eye(5, dtype=np.float32), np.ones((25, 1), np.float32))
    bo40 = np.zeros((125, 8, 40), np.float32)
    for g in range(8):
        bo40[:, g, 5 * g:5 * g + 5] = bo125
    cb[0:125, C_BO40:C_BO40 + 320] = bo40.reshape(125, 320)
    cb[0:125, C_BDM:C_BDM + 125] = np.kron(
        np.eye(5, dtype=np.float32), np.ones((25, 25), np.float32))
    cb[:, C_ONES] = 1.0
    cb[0, C_ONES40:C_ONES40 + 40] = 1.0

    cf = np.zeros((128, NCF), np.float32)
    indB = np.zeros((128, 8, 40), np.float32)
    for s in range(8):
        for p in range(128):
            l = s * 128 + p
            if l < TB:
                indB[p, s, l // 25] = 1.0
    cf[:, F_IND:F_IND + 320] = indB.reshape(128, 320)
    cf[:, F_ID:F_ID + 128] = np.eye(128)
    return (cb.astype(ml_dtypes.bfloat16), cf)


def kernel(features_a, features_b, W):
    features_a = np.asarray(features_a, np.float32)
    features_b = np.asarray(features_b, np.float32)
    W = np.asarray(W, np.float32)
    f8 = ml_dtypes.float8_e4m3

    nq_total = features_b.shape[0]
    fbp = np.zeros((N_CORES * NQ, 640, LB), np.float32)
    fbp[:nq_total] = features_b
    fb_t = np.ascontiguousarray(fbp.transpose(1, 0, 2)).astype(f8)
    fa_pad = np.zeros((640, NW, 128), np.float32)
    fa_pad[:, :, :LA] = features_a.transpose(1, 0, 2)
    fa_t = np.ascontiguousarray(fa_pad.reshape(640, TAP)).astype(f8)
    wt = np.ascontiguousarray(W.T * WS).astype(f8)
    cb, cf = _consts()

    in_maps = []
    for c in range(N_CORES):
        m = {
            "fa": fa_t,
            "fb": np.ascontiguousarray(
                fb_t[:, c * NQ:(c + 1) * NQ, :]).reshape(640, TB),
            "wt": wt,
            "cb": cb,
            "cf": cf,
        }
        in_maps.append(m)

    nc = _get_nc()
    res = run_bass_kernel_spmd(nc, in_maps, core_ids=list(range(N_CORES)))

    sq = np.zeros((N_CORES * NQ, NW), np.float32)
    qs = np.zeros((N_CORES * NQ, NW), np.float32)
    for c in range(N_CORES):
        sq[c * NQ:(c + 1) * NQ] = res.results[c]["sq"]
        qs[c * NQ:(c + 1) * NQ] = res.results[c]["qs"]
    return sq[:nq_total], qs[:nq_total]
